# revision 6
# baseline (speedup 1.0000x reference)
"""2-layer LSTM decoder for trn2 — single-core, hardware-looped.

The execution backend (axon/fake_nrt) charges wall time per STATIC
instruction (~55us each) plus ~20us per executed loop-iteration
boundary; instruction size barely matters and cores emulate in
parallel.  So the whole T=168 recurrence runs on ONE core inside a
For_i(0,T) loop whose body is static (~280 instructions total), with
inner loops 2x-unrolled (For_i step=2) to halve iteration boundaries:

- Weights live in DRAM, pre-grouped per loop iteration so each
  iteration issues ONE DMA (L0: 4 chunks/quad, L1: 2 chunks/pair) into
  a fixed SBUF tile (dynamic DRAM APs are allowed; matmul stationary
  APs must be static, so the dynamic index rides the DMA).
- Gate psums [128gate x 512batch] accumulate over K-chunks; activations
  write sig tiles at the dynamic j offset (allowed on act outputs).
- L1's bias is a 17th K-chunk: stationary [1,128] b1 slice against a
  constant ones [1,512] row.  L0's bias rides the xk ones row.
- Teacher forcing is data-driven: host precomputes knb row0 (y*tf) and
  tfc (1-tf); in-loop, xk_row0 += tfc[t] * pred(t-1), so no per-step
  branching and the loop body stays static.
"""
import numpy as np
import ml_dtypes

import concourse.bass as bass
import concourse.mybir as mybir
import concourse.tile as tile
from concourse import bacc

F32 = mybir.dt.float32
BF16 = mybir.dt.bfloat16
AF = mybir.ActivationFunctionType
ALU = mybir.AluOpType

B, T_FULL, F, H, GE = 512, 168, 32, 1024, 16
N_CORES = 1
NJ = 8                     # hidden chunks (H/128)
KX = 50                    # xk rows: prev_y(1) + known(32) + gv(16) + ones(1)
K0 = 9                     # L0 K-chunks per gate: 8 h + 1 xk
K1 = 17                    # L1 K-chunks per gate: 8 h1 + 8 h0new + 1 bias


def prep_host(inputs, T):
    inp = {k: np.asarray(v) for k, v in inputs.items()}
    gv = inp["group_emb"][inp["group_ids"].astype(np.int64)]       # (B, GE)
    b0 = (inp["b_ih0"] + inp["b_hh0"]).astype(np.float32)          # (4096,)
    b1 = (inp["b_ih1"] + inp["b_hh1"]).astype(np.float32)
    Whh0 = inp["W_hh0"].astype(np.float32)                         # (4096, 1024)
    Whh1 = inp["W_hh1"].astype(np.float32)
    Wih1 = inp["W_ih1"].astype(np.float32)
    Wih0 = inp["W_ih0"].astype(np.float32)                         # (4096, 49)

    # w0[j, p, (X*9+k)*128+m]: k<8 -> Whh0[X*1024+j*128+m, k*128+p],
    #                          k=8 -> rows 0:49 Wih0, row 49 b0.
    A = np.zeros((NJ, 128, 4, K0, 128), np.float32)
    A[:, :, :, :8, :] = Whh0.reshape(4, NJ, 128, 8, 128).transpose(1, 4, 0, 3, 2)
    A[:, :49, :, 8, :] = Wih0.reshape(4, NJ, 128, 49).transpose(1, 3, 0, 2)
    A[:, 49, :, 8, :] = b0.reshape(4, NJ, 128).transpose(1, 0, 2)
    w0 = np.ascontiguousarray(A.reshape(NJ, 128, 4 * K0 * 128)).astype(
        ml_dtypes.bfloat16)

    # w1[j, p, (X*17+k)*128+m]: k<8 Whh1, k in 8..15 Wih1, k=16 row0 b1.
    Bm = np.zeros((NJ, 128, 4, K1, 128), np.float32)
    Bm[:, :, :, :8, :] = Whh1.reshape(4, NJ, 128, 8, 128).transpose(1, 4, 0, 3, 2)
    Bm[:, :, :, 8:16, :] = Wih1.reshape(4, NJ, 128, 8, 128).transpose(1, 4, 0, 3, 2)
    Bm[:, 0, :, 16, :] = b1.reshape(4, NJ, 128).transpose(1, 0, 2)
    w1 = np.ascontiguousarray(Bm.reshape(NJ, 128, 4 * K1 * 128)).astype(
        ml_dtypes.bfloat16)

    wp = inp["W_proj"].astype(np.float32)[0]                       # (1024,)
    wpT = np.ascontiguousarray(wp.reshape(NJ, 128).T).astype(ml_dtypes.bfloat16)

    y = inp["target_y"].astype(np.float32)[:, :, 0]                # (B, T)
    tf = np.asarray(inp["tf_mask"]).reshape(-1).astype(np.float32)[:T]
    knb = np.zeros((T, KX, B), np.float32)
    knb[0, 0] = inp["last_enc_consumption"].astype(np.float32)[:, 0]
    for t in range(1, T):
        knb[t, 0] = tf[t - 1] * y[:, t - 1]
    knb[:, 1:33] = inp["dec_known"].astype(np.float32)[:, :T, :].transpose(1, 2, 0)
    knb[:, 33:49] = gv.T[None]
    knb[:, 49] = 1.0
    knb = knb.astype(ml_dtypes.bfloat16)

    tfc = np.zeros((T, 1, B), np.float32)
    for t in range(1, T):
        tfc[t, 0] = 1.0 - tf[t - 1]
    tfc = tfc.astype(ml_dtypes.bfloat16)

    # group weight blocks so each loop iteration is ONE DMA:
    w0q = np.zeros((5, 128, 4, 4 * K0 * 128), ml_dtypes.bfloat16)
    for jv in (0, 4):
        w0q[jv] = w0[jv:jv + 4].transpose(1, 0, 2)
    w1q = np.zeros((7, 128, 2, 4 * K1 * 128), ml_dtypes.bfloat16)
    for jv in (0, 2, 4, 6):
        w1q[jv] = w1[jv:jv + 2].transpose(1, 0, 2)

    def st(a):          # (B, H) -> [128, NJ, B]
        return np.ascontiguousarray(
            a.astype(np.float32).reshape(B, NJ, 128).transpose(2, 1, 0))

    shared = dict(
        w0=w0q, w1=w1q, wpT=wpT, knb=knb, tfc=tfc,
        h0i=st(inp["h0"][0]).astype(ml_dtypes.bfloat16),
        h1i=st(inp["h0"][1]).astype(ml_dtypes.bfloat16),
        c0i=st(inp["c0"][0]),
        c1i=st(inp["c0"][1]),
    )
    per_core = [dict() for _ in range(N_CORES)]
    tf_mask = [int(v) for v in np.asarray(inp["tf_mask"]).reshape(-1)][:T]
    b_proj = float(np.asarray(inp["b_proj"]).reshape(-1)[0])
    return shared, per_core, tf_mask, b_proj


def build_module(T, tf_mask, b_proj, rep=1):
    nc = bacc.Bacc(target_bir_lowering=False)

    w0_d = nc.dram_tensor("w0", [5, 128, 4, 4 * K0 * 128], BF16, kind="ExternalInput")
    w1_d = nc.dram_tensor("w1", [7, 128, 2, 4 * K1 * 128], BF16, kind="ExternalInput")
    wpT_d = nc.dram_tensor("wpT", [128, NJ], BF16, kind="ExternalInput")
    knb_d = nc.dram_tensor("knb", [T, KX, B], BF16, kind="ExternalInput")
    tfc_d = nc.dram_tensor("tfc", [T, 1, B], BF16, kind="ExternalInput")
    h0i_d = nc.dram_tensor("h0i", [128, NJ, B], BF16, kind="ExternalInput")
    h1i_d = nc.dram_tensor("h1i", [128, NJ, B], BF16, kind="ExternalInput")
    c0i_d = nc.dram_tensor("c0i", [128, NJ, B], F32, kind="ExternalInput")
    c1i_d = nc.dram_tensor("c1i", [128, NJ, B], F32, kind="ExternalInput")
    out_d = nc.dram_tensor("out", [T, 1, B], F32, kind="ExternalOutput")

    AFS = [AF.Sigmoid, AF.Sigmoid, AF.Tanh, AF.Sigmoid]   # i, f, g, o

    with tile.TileContext(nc) as tc:
        with tc.tile_pool(name="const", bufs=1) as const, \
             tc.tile_pool(name="state", bufs=1) as stp, \
             tc.tile_pool(name="act", bufs=1) as actp, \
             tc.tile_pool(name="wld", bufs=1) as wld, \
             tc.tile_pool(name="io", bufs=2) as iop, \
             tc.tile_pool(name="gps", bufs=1, space="PSUM") as gpsum:

            wpT = const.tile([128, NJ], BF16)
            nc.sync.dma_start(out=wpT[:], in_=wpT_d[:])
            ones = const.tile([1, B], BF16)
            nc.vector.memset(ones[:], 1.0)

            def rep_body(r):
                h0f = stp.tile([128, NJ, B], BF16, tag="h0f", name=f"h0_{r}")
                nc.sync.dma_start(out=h0f[:], in_=h0i_d[:])
                h1f = stp.tile([128, NJ, B], BF16, tag="h1f", name=f"h1_{r}")
                nc.sync.dma_start(out=h1f[:], in_=h1i_d[:])
                c0 = stp.tile([128, NJ, B], F32, tag="c0", name=f"c0_{r}")
                nc.sync.dma_start(out=c0[:], in_=c0i_d[:])
                c1 = stp.tile([128, NJ, B], F32, tag="c1", name=f"c1_{r}")
                nc.sync.dma_start(out=c1[:], in_=c1i_d[:])
                pred = stp.tile([1, B], F32, tag="pred", name=f"pred_{r}")
                nc.vector.memset(pred[:], 0.0)

                sig = [actp.tile([128, NJ, B], BF16, tag=f"sig{X}",
                                 name=f"sig{X}_{r}") for X in range(4)]

                def cell(c_cur, hf, lab):
                    tmpf = actp.tile([128, NJ, B], F32, tag="tmpf",
                                     name=f"tf_{lab}")
                    nc.vector.tensor_tensor(out=tmpf[:], in0=sig[1][:],
                                            in1=c_cur[:], op=ALU.mult)
                    tmpb = actp.tile([128, NJ, B], BF16, tag="tmpb",
                                     name=f"tb_{lab}")
                    nc.vector.tensor_tensor(out=tmpb[:], in0=sig[0][:],
                                            in1=sig[2][:], op=ALU.mult)
                    nc.vector.tensor_tensor(out=c_cur[:], in0=tmpf[:],
                                            in1=tmpb[:], op=ALU.add)
                    tanc = actp.tile([128, NJ, B], BF16, tag="tanc",
                                     name=f"tc_{lab}")
                    nc.scalar.activation(tanc[:], c_cur[:], AF.Tanh)
                    nc.vector.tensor_tensor(out=hf[:], in0=sig[3][:],
                                            in1=tanc[:], op=ALU.mult)

                with tc.For_i(0, T) as it:
                    # ---- assemble x(t): load block, add tfc*pred into row 0
                    xk = iop.tile([KX, B], BF16, tag="xk")
                    nc.sync.dma_start(out=xk[:], in_=knb_d[it])
                    tfr = iop.tile([1, B], BF16, tag="tfr")
                    nc.sync.dma_start(out=tfr[:], in_=tfc_d[it])
                    fb = iop.tile([1, B], BF16, tag="fb")
                    nc.vector.tensor_tensor(out=fb[:], in0=pred[:],
                                            in1=tfr[:], op=ALU.mult)
                    nc.vector.tensor_tensor(out=xk[0:1, :], in0=xk[0:1, :],
                                            in1=fb[:], op=ALU.add)

                    # ---- layer 0: gates for hidden chunks (jv, jv+1)
                    with tc.For_i(0, NJ, 4) as jv:
                        w0a = wld.tile([128, 4, 4 * K0 * 128], BF16, tag="w0a")
                        nc.sync.dma_start(out=w0a[:], in_=w0_d[jv])
                        for s in range(4):
                            w0c = w0a[:, s]
                            for X in range(4):
                                g = gpsum.tile([128, B], F32, tag=f"g{X}")
                                for k in range(8):
                                    nc.tensor.matmul(
                                        g[:], w0c[:, (X * K0 + k) * 128:
                                                  (X * K0 + k + 1) * 128],
                                        h0f[:, k, :], start=(k == 0), stop=False)
                                nc.tensor.matmul(
                                    g[:], w0c[0:KX, (X * K0 + 8) * 128:
                                              (X * K0 + 9) * 128],
                                    xk[:], start=False, stop=True)
                                nc.scalar.activation(sig[X][:, jv + s], g[:],
                                                     AFS[X])
                    cell(c0, h0f, "l0")

                    # ---- layer 1
                    with tc.For_i(0, NJ, 2) as jv:
                        w1a = wld.tile([128, 2, 4 * K1 * 128], BF16, tag="w1a")
                        nc.sync.dma_start(out=w1a[:], in_=w1_d[jv])
                        for s in range(2):
                            w1c = w1a[:, s]
                            for X in range(4):
                                g = gpsum.tile([128, B], F32, tag=f"g{X}")
                                for k in range(8):
                                    nc.tensor.matmul(
                                        g[:], w1c[:, (X * K1 + k) * 128:
                                                  (X * K1 + k + 1) * 128],
                                        h1f[:, k, :], start=(k == 0), stop=False)
                                for k in range(8, 16):
                                    nc.tensor.matmul(
                                        g[:], w1c[:, (X * K1 + k) * 128:
                                                  (X * K1 + k + 1) * 128],
                                        h0f[:, k - 8, :], start=False, stop=False)
                                nc.tensor.matmul(
                                    g[:], w1c[0:1, (X * K1 + 16) * 128:
                                              (X * K1 + 17) * 128],
                                    ones[:], start=False, stop=True)
                                nc.scalar.activation(sig[X][:, jv + s], g[:],
                                                     AFS[X])
                    cell(c1, h1f, "l1")

                    # ---- pred(t) = wp . h1 + b_proj
                    pp = gpsum.tile([1, B], F32, tag="pp")
                    for k in range(NJ):
                        nc.tensor.matmul(pp[:], wpT[:, k:k + 1], h1f[:, k, :],
                                         start=(k == 0), stop=(k == NJ - 1))
                    nc.vector.tensor_scalar_add(pred[:], pp[:], b_proj)
                    nc.sync.dma_start(out=out_d[it], in_=pred[:])

            for r in range(rep):
                rep_body(r)

    nc.finalize()
    return nc


def kernel(**inputs):
    import time
    from concourse.bass_utils import run_bass_kernel_spmd
    T = T_FULL
    shared, per_core, tf_mask, b_proj = prep_host(inputs, T)
    nc = build_module(T, tf_mask, b_proj)
    in_maps = []
    for c in range(N_CORES):
        m = dict(shared)
        m.update(per_core[c])
        in_maps.append(m)
    res = None
    for attempt in range(3):
        try:
            res = run_bass_kernel_spmd(nc, in_maps, list(range(N_CORES)))
            break
        except Exception:
            if attempt == 2:
                raise
            time.sleep(5)
    ob = res.results[0]["out"].astype(np.float32)      # (T, 1, B)
    return np.ascontiguousarray(ob[:, 0, :].T)[:, :, None]  # (B, T, 1)


# revision 7
# speedup vs baseline: 1.2027x; 1.2027x over previous
"""2-layer LSTM decoder for trn2 — single-core, hardware-looped.

The execution backend (axon/fake_nrt) charges wall time per STATIC
instruction (~55us each) plus ~20us per executed loop-iteration
boundary; instruction size barely matters and cores emulate in
parallel.  So the whole T=168 recurrence runs on ONE core inside a
For_i(0,T) loop whose body is static (~350 instructions total), with
the L0 inner loop 4x-unrolled (step=4) and L1 2x-unrolled (step=2),
7 loop-iteration boundaries per step:

- Weights live in DRAM, pre-grouped per loop iteration so each
  iteration issues ONE DMA (L0: 4 chunks/quad, L1: 2 chunks/pair) into
  a fixed SBUF tile (dynamic DRAM APs are allowed; matmul stationary
  APs must be static, so the dynamic index rides the DMA).
- Gate psums [128gate x 512batch] accumulate over K-chunks; activations
  write sig tiles at the dynamic j offset (allowed on act outputs).
- L1's bias is a 17th K-chunk: stationary [1,128] b1 slice against a
  constant ones [1,512] row.  L0's bias rides the xk ones row.
- Teacher forcing is data-driven: host precomputes knb row0 (y*tf) and
  tfc (1-tf); in-loop, xk_row0 += tfc[t] * pred(t-1), so no per-step
  branching and the loop body stays static.
"""
import numpy as np
import ml_dtypes

import concourse.bass as bass
import concourse.mybir as mybir
import concourse.tile as tile
from concourse import bacc

F32 = mybir.dt.float32
BF16 = mybir.dt.bfloat16
AF = mybir.ActivationFunctionType
ALU = mybir.AluOpType

B, T_FULL, F, H, GE = 512, 168, 32, 1024, 16
N_CORES = 1
NJ = 8                     # hidden chunks (H/128)
KX = 50                    # xk rows: prev_y(1) + known(32) + gv(16) + ones(1)
K0 = 9                     # L0 K-chunks per gate: 8 h + 1 xk
K1 = 17                    # L1 K-chunks per gate: 8 h1 + 8 h0new + 1 bias


def prep_host(inputs, T):
    inp = {k: np.asarray(v) for k, v in inputs.items()}
    gv = inp["group_emb"][inp["group_ids"].astype(np.int64)]       # (B, GE)
    b0 = (inp["b_ih0"] + inp["b_hh0"]).astype(np.float32)          # (4096,)
    b1 = (inp["b_ih1"] + inp["b_hh1"]).astype(np.float32)
    Whh0 = inp["W_hh0"].astype(np.float32)                         # (4096, 1024)
    Whh1 = inp["W_hh1"].astype(np.float32)
    Wih1 = inp["W_ih1"].astype(np.float32)
    Wih0 = inp["W_ih0"].astype(np.float32)                         # (4096, 49)

    # w0[j, p, (X*9+k)*128+m]: k<8 -> Whh0[X*1024+j*128+m, k*128+p],
    #                          k=8 -> rows 0:49 Wih0, row 49 b0.
    A = np.zeros((NJ, 128, 4, K0, 128), np.float32)
    A[:, :, :, :8, :] = Whh0.reshape(4, NJ, 128, 8, 128).transpose(1, 4, 0, 3, 2)
    A[:, :49, :, 8, :] = Wih0.reshape(4, NJ, 128, 49).transpose(1, 3, 0, 2)
    A[:, 49, :, 8, :] = b0.reshape(4, NJ, 128).transpose(1, 0, 2)
    w0 = np.ascontiguousarray(A.reshape(NJ, 128, 4 * K0 * 128)).astype(
        ml_dtypes.bfloat16)

    # w1[j, p, (X*17+k)*128+m]: k<8 Whh1, k in 8..15 Wih1, k=16 row0 b1.
    Bm = np.zeros((NJ, 128, 4, K1, 128), np.float32)
    Bm[:, :, :, :8, :] = Whh1.reshape(4, NJ, 128, 8, 128).transpose(1, 4, 0, 3, 2)
    Bm[:, :, :, 8:16, :] = Wih1.reshape(4, NJ, 128, 8, 128).transpose(1, 4, 0, 3, 2)
    Bm[:, 0, :, 16, :] = b1.reshape(4, NJ, 128).transpose(1, 0, 2)
    w1 = np.ascontiguousarray(Bm.reshape(NJ, 128, 4 * K1 * 128)).astype(
        ml_dtypes.bfloat16)

    wp = inp["W_proj"].astype(np.float32)[0]                       # (1024,)
    wpT = np.ascontiguousarray(wp.reshape(NJ, 128).T).astype(ml_dtypes.bfloat16)

    y = inp["target_y"].astype(np.float32)[:, :, 0]                # (B, T)
    tf = np.asarray(inp["tf_mask"]).reshape(-1).astype(np.float32)[:T]
    knb = np.zeros((T, KX, B), np.float32)
    knb[0, 0] = inp["last_enc_consumption"].astype(np.float32)[:, 0]
    for t in range(1, T):
        knb[t, 0] = tf[t - 1] * y[:, t - 1]
    knb[:, 1:33] = inp["dec_known"].astype(np.float32)[:, :T, :].transpose(1, 2, 0)
    knb[:, 33:49] = gv.T[None]
    knb[:, 49] = 1.0
    knb = knb.astype(ml_dtypes.bfloat16)

    tfc = np.zeros((T, 1, B), np.float32)
    for t in range(1, T):
        tfc[t, 0] = 1.0 - tf[t - 1]
    tfc = tfc.astype(ml_dtypes.bfloat16)

    # group weight blocks so each loop iteration is ONE DMA:
    w0q = np.zeros((5, 128, 4, 4 * K0 * 128), ml_dtypes.bfloat16)
    for jv in (0, 4):
        w0q[jv] = w0[jv:jv + 4].transpose(1, 0, 2)
    w1q = np.zeros((7, 128, 2, 4 * K1 * 128), ml_dtypes.bfloat16)
    for jv in (0, 2, 4, 6):
        w1q[jv] = w1[jv:jv + 2].transpose(1, 0, 2)

    def st(a):          # (B, H) -> [128, NJ, B]
        return np.ascontiguousarray(
            a.astype(np.float32).reshape(B, NJ, 128).transpose(2, 1, 0))

    shared = dict(
        w0=w0q, w1=w1q, wpT=wpT, knb=knb, tfc=tfc,
        h0i=st(inp["h0"][0]).astype(ml_dtypes.bfloat16),
        h1i=st(inp["h0"][1]).astype(ml_dtypes.bfloat16),
        c0i=st(inp["c0"][0]),
        c1i=st(inp["c0"][1]),
    )
    per_core = [dict() for _ in range(N_CORES)]
    tf_mask = [int(v) for v in np.asarray(inp["tf_mask"]).reshape(-1)][:T]
    b_proj = float(np.asarray(inp["b_proj"]).reshape(-1)[0])
    return shared, per_core, tf_mask, b_proj


def build_module(T, tf_mask, b_proj, rep=1):
    nc = bacc.Bacc(target_bir_lowering=False)

    w0_d = nc.dram_tensor("w0", [5, 128, 4, 4 * K0 * 128], BF16, kind="ExternalInput")
    w1_d = nc.dram_tensor("w1", [7, 128, 2, 4 * K1 * 128], BF16, kind="ExternalInput")
    wpT_d = nc.dram_tensor("wpT", [128, NJ], BF16, kind="ExternalInput")
    knb_d = nc.dram_tensor("knb", [T, KX, B], BF16, kind="ExternalInput")
    tfc_d = nc.dram_tensor("tfc", [T, 1, B], BF16, kind="ExternalInput")
    h0i_d = nc.dram_tensor("h0i", [128, NJ, B], BF16, kind="ExternalInput")
    h1i_d = nc.dram_tensor("h1i", [128, NJ, B], BF16, kind="ExternalInput")
    c0i_d = nc.dram_tensor("c0i", [128, NJ, B], F32, kind="ExternalInput")
    c1i_d = nc.dram_tensor("c1i", [128, NJ, B], F32, kind="ExternalInput")
    out_d = nc.dram_tensor("out", [T, 1, B], F32, kind="ExternalOutput")

    AFS = [AF.Sigmoid, AF.Sigmoid, AF.Tanh, AF.Sigmoid]   # i, f, g, o

    with tile.TileContext(nc) as tc:
        with tc.tile_pool(name="const", bufs=1) as const, \
             tc.tile_pool(name="state", bufs=1) as stp, \
             tc.tile_pool(name="act", bufs=1) as actp, \
             tc.tile_pool(name="wld", bufs=1) as wld, \
             tc.tile_pool(name="io", bufs=2) as iop, \
             tc.tile_pool(name="gps", bufs=1, space="PSUM") as gpsum:

            wpT = const.tile([128, NJ], BF16)
            nc.sync.dma_start(out=wpT[:], in_=wpT_d[:])
            ones = const.tile([1, B], BF16)
            nc.vector.memset(ones[:], 1.0)

            def rep_body(r):
                h0f = stp.tile([128, NJ, B], BF16, tag="h0f", name=f"h0_{r}")
                nc.sync.dma_start(out=h0f[:], in_=h0i_d[:])
                h1f = stp.tile([128, NJ, B], BF16, tag="h1f", name=f"h1_{r}")
                nc.sync.dma_start(out=h1f[:], in_=h1i_d[:])
                c0 = stp.tile([128, NJ, B], F32, tag="c0", name=f"c0_{r}")
                nc.sync.dma_start(out=c0[:], in_=c0i_d[:])
                c1 = stp.tile([128, NJ, B], F32, tag="c1", name=f"c1_{r}")
                nc.sync.dma_start(out=c1[:], in_=c1i_d[:])
                pred = stp.tile([1, B], F32, tag="pred", name=f"pred_{r}")
                nc.vector.memset(pred[:], 0.0)

                sig = [actp.tile([128, NJ, B], BF16, tag=f"sig{X}",
                                 name=f"sig{X}_{r}") for X in range(4)]

                def cell(c_cur, hf, lab):
                    tmpf = actp.tile([128, NJ, B], F32, tag="tmpf",
                                     name=f"tf_{lab}")
                    nc.vector.tensor_tensor(out=tmpf[:], in0=sig[1][:],
                                            in1=c_cur[:], op=ALU.mult)
                    tmpb = actp.tile([128, NJ, B], BF16, tag="tmpb",
                                     name=f"tb_{lab}")
                    nc.vector.tensor_tensor(out=tmpb[:], in0=sig[0][:],
                                            in1=sig[2][:], op=ALU.mult)
                    nc.vector.tensor_tensor(out=c_cur[:], in0=tmpf[:],
                                            in1=tmpb[:], op=ALU.add)
                    tanc = actp.tile([128, NJ, B], BF16, tag="tanc",
                                     name=f"tc_{lab}")
                    nc.scalar.activation(tanc[:], c_cur[:], AF.Tanh)
                    nc.vector.tensor_tensor(out=hf[:], in0=sig[3][:],
                                            in1=tanc[:], op=ALU.mult)

                with tc.For_i(0, T) as it:
                    # ---- assemble x(t): load block, add tfc*pred into row 0
                    xk = iop.tile([KX, B], BF16, tag="xk")
                    nc.sync.dma_start(out=xk[:], in_=knb_d[it])
                    tfr = iop.tile([1, B], BF16, tag="tfr")
                    nc.sync.dma_start(out=tfr[:], in_=tfc_d[it])
                    fb = iop.tile([1, B], BF16, tag="fb")
                    nc.vector.tensor_tensor(out=fb[:], in0=pred[:],
                                            in1=tfr[:], op=ALU.mult)
                    nc.vector.tensor_tensor(out=xk[0:1, :], in0=xk[0:1, :],
                                            in1=fb[:], op=ALU.add)

                    # ---- layer 0: gates for hidden chunks (jv, jv+1)
                    with tc.For_i(0, NJ, 4) as jv:
                        w0a = wld.tile([128, 4, 4 * K0 * 128], BF16, tag="w0a")
                        nc.sync.dma_start(out=w0a[:], in_=w0_d[jv])
                        for s in range(4):
                            w0c = w0a[:, s]
                            for X in range(4):
                                g = gpsum.tile([128, B], F32, tag=f"g{X}")
                                for k in range(8):
                                    nc.tensor.matmul(
                                        g[:], w0c[:, (X * K0 + k) * 128:
                                                  (X * K0 + k + 1) * 128],
                                        h0f[:, k, :], start=(k == 0), stop=False)
                                nc.tensor.matmul(
                                    g[:], w0c[0:KX, (X * K0 + 8) * 128:
                                              (X * K0 + 9) * 128],
                                    xk[:], start=False, stop=True)
                                nc.scalar.activation(sig[X][:, jv + s], g[:],
                                                     AFS[X])
                    cell(c0, h0f, "l0")

                    # ---- layer 1
                    with tc.For_i(0, NJ, 2) as jv:
                        w1a = wld.tile([128, 2, 4 * K1 * 128], BF16, tag="w1a")
                        nc.sync.dma_start(out=w1a[:], in_=w1_d[jv])
                        for s in range(2):
                            w1c = w1a[:, s]
                            for X in range(4):
                                g = gpsum.tile([128, B], F32, tag=f"g{X}")
                                for k in range(8):
                                    nc.tensor.matmul(
                                        g[:], w1c[:, (X * K1 + k) * 128:
                                                  (X * K1 + k + 1) * 128],
                                        h1f[:, k, :], start=(k == 0), stop=False)
                                for k in range(8, 16):
                                    nc.tensor.matmul(
                                        g[:], w1c[:, (X * K1 + k) * 128:
                                                  (X * K1 + k + 1) * 128],
                                        h0f[:, k - 8, :], start=False, stop=False)
                                nc.tensor.matmul(
                                    g[:], w1c[0:1, (X * K1 + 16) * 128:
                                              (X * K1 + 17) * 128],
                                    ones[:], start=False, stop=True)
                                nc.scalar.activation(sig[X][:, jv + s], g[:],
                                                     AFS[X])
                    cell(c1, h1f, "l1")

                    # ---- pred(t) = wp . h1 + b_proj
                    pp = gpsum.tile([1, B], F32, tag="pp")
                    for k in range(NJ):
                        nc.tensor.matmul(pp[:], wpT[:, k:k + 1], h1f[:, k, :],
                                         start=(k == 0), stop=(k == NJ - 1))
                    nc.vector.tensor_scalar_add(pred[:], pp[:], b_proj)
                    nc.sync.dma_start(out=out_d[it], in_=pred[:])

            for r in range(rep):
                rep_body(r)

    nc.finalize()
    return nc


def kernel(**inputs):
    import time
    from concourse.bass_utils import run_bass_kernel_spmd
    T = T_FULL
    shared, per_core, tf_mask, b_proj = prep_host(inputs, T)
    nc = build_module(T, tf_mask, b_proj)
    in_maps = []
    for c in range(N_CORES):
        m = dict(shared)
        m.update(per_core[c])
        in_maps.append(m)
    res = None
    for attempt in range(3):
        try:
            res = run_bass_kernel_spmd(nc, in_maps, list(range(N_CORES)))
            break
        except Exception:
            if attempt == 2:
                raise
            time.sleep(5)
    ob = res.results[0]["out"].astype(np.float32)      # (T, 1, B)
    return np.ascontiguousarray(ob[:, 0, :].T)[:, :, None]  # (B, T, 1)
